# revision 11
# baseline (speedup 1.0000x reference)
"""Multi-head self-attention (B=2, T=2048, d_model=1024, 16 heads, causal)
on 8 trn2 NeuronCores.

Sharding: core c -> batch b=c//4, head-group g=c%4 (4 heads, d_model slice
of 256). Each core computes its heads' attention and a partial wo
projection [2048, 1024]; host sums the 4 partials per batch and adds bo.

Per-core pipeline (all matmul inputs bf16, fp32 PSUM accumulation):
  QT[dq,t] = (wq_s @ x^T)*0.125 + bq*0.125   (scale folded into weights)
  KT[dk,t] = wk_s @ x^T + bk
  V[t,dv]  = x @ wv_s^T + bv
  per head pair, per q-block j (512 wide):
    S^T[k,q] = K_h @ Q_h^T   (K=64 contraction, heads packed at partition
                              bases 0/64 -> concurrent row-group matmuls)
    E = exp(S^T)             (ACT, groups of 4 PSUM banks = 2 kt x 2 heads)
    diag tiles: E *= upper-tri mask
    O^T_aug = V_aug^T @ E    (V_aug = [64 ones cols | V_h cols] so PSUM rows
                              0-63 = replicated rowsums, rows 64-127 = O^T)
    OT_all = O^T * recip(rowsum)  (DVE reciprocal + tensor_tensor mul)
  P = OT_all^T @ wo_s^T      (partial output, fp32 out)
"""
import sys
sys.path.insert(0, "/opt/trn_rl_repo")
import numpy as np
import ml_dtypes

import concourse.bass as bass
import concourse.bacc as bacc
import concourse.tile as tile
import concourse.mybir as mybir
from concourse import bass_utils

BF16 = mybir.dt.bfloat16
F32 = mybir.dt.float32
EXP = mybir.ActivationFunctionType.Exp

T = 2048          # sequence length
DM = 1024         # d_model
DS = 256          # per-core d_model slice (4 heads x 64)
HD = 64           # head dim
NH = 4            # heads per core
KT128 = 16        # k tiles of 128 over T
QB = 512          # q block width
NJ = T // QB      # 4 q blocks
NCORES = 8

_CACHE = {}


def _build():
    nc = bacc.Bacc("TRN2", target_bir_lowering=False, debug=False,
                   enable_asserts=False, num_devices=NCORES)
    dram = {}
    for name, shape, dt in [
        ("xt", [DM, T], BF16),        # x[b]^T
        ("wqt", [DM, DS], BF16),      # wq.T[:, slice] * 0.125
        ("wkt", [DM, DS], BF16),
        ("wvt", [DM, DS], BF16),
        ("wot", [DS, DM], BF16),      # wo[:, slice].T
        ("bqc", [128, 2], F32),       # bq*0.125 as [128, m] columns
        ("bkc", [128, 2], F32),
        ("bv", [1, DS], BF16),
        ("tri", [128, 128], BF16),    # upper-tri (incl diag) ones
    ]:
        dram[name] = nc.dram_tensor(name, shape, dt, kind="ExternalInput").ap()
    p_out = nc.dram_tensor("p_out", [T, DM], F32, kind="ExternalOutput").ap()

    with tile.TileContext(nc) as tc:
        with tc.tile_pool(name="persist", bufs=1) as pp, \
             tc.tile_pool(name="epool", bufs=2) as ep, \
             tc.tile_pool(name="outp", bufs=4) as op, \
             tc.tile_pool(name="bcp", bufs=2) as bp, \
             tc.tile_pool(name="misc_ps", bufs=2, space="PSUM") as mp, \
             tc.tile_pool(name="st_ps", bufs=1, space="PSUM") as sp, \
             tc.tile_pool(name="ot_ps", bufs=2, space="PSUM") as tp:

            # ---- persistent SBUF ----
            xt = pp.tile([128, 8, T], BF16, name="xt")        # [p, kt8, t]
            wqt = pp.tile([128, 8, DS], BF16, name="wqt")
            wkt = pp.tile([128, 8, DS], BF16, name="wkt")
            wvt = pp.tile([128, 8, DS], BF16, name="wvt")
            wot = pp.tile([128, 2, DM], BF16, name="wot")
            qt = pp.tile([128, 2, T], BF16, name="qt")        # [p, dq-tile, t]
            kt = pp.tile([128, 2, T], BF16, name="kt")
            vaug = pp.tile([128, KT128, 512], BF16, name="vaug")
            ot_all = pp.tile([128, 2, T], BF16, name="ot_all")
            ones_row = pp.tile([1, 512], BF16, name="ones_row")
            bqc = pp.tile([128, 2], F32, name="bqc")
            bkc = pp.tile([128, 2], F32, name="bkc")
            bv_r = pp.tile([1, DS], BF16, name="bv_r")
            tri = pp.tile([128, 128], BF16, name="tri")

            nc.gpsimd.memset(ones_row, 1.0)
            # V_aug head block h: cols [128h, 128h+64) ones, [128h+64, +128) V
            for h in range(NH):
                nc.gpsimd.memset(vaug[:, :, 128 * h:128 * h + HD], 1.0)

            xt_dram = dram["xt"].rearrange("(kt p) t -> p kt t", p=128)
            # x t-slice 0 goes first on the sync queue so matmuls start early;
            # weights go on the gpsimd queue in parallel.
            for k in range(8):
                nc.sync.dma_start(out=xt[:, k, 0:512], in_=xt_dram[:, k, 0:512])
            for k in range(8):
                for w_sb, w_nm in [(wqt, "wqt"), (wkt, "wkt"), (wvt, "wvt")]:
                    nc.gpsimd.dma_start(
                        out=w_sb[:, k, :],
                        in_=dram[w_nm].rearrange("(kt p) d -> p kt d", p=128)[:, k, :])
            nc.gpsimd.dma_start(out=bqc, in_=dram["bqc"])
            nc.gpsimd.dma_start(out=bkc, in_=dram["bkc"])
            nc.gpsimd.dma_start(out=bv_r, in_=dram["bv"])
            nc.gpsimd.dma_start(out=tri, in_=dram["tri"])
            nc.gpsimd.dma_start(
                out=wot, in_=dram["wot"].rearrange("(kt p) d -> p kt d", p=128))

            # ---- projections, streamed by t-slice of 512 ----
            for ts in range(4):
                t0 = ts * 512
                if ts > 0:
                    for k in range(8):
                        nc.sync.dma_start(out=xt[:, k, t0:t0 + 512],
                                          in_=xt_dram[:, k, t0:t0 + 512])
                # QT / KT: out [dq 128, t 512]; bias fused into ACT copy
                for w_sb, b_c, dst in ((wqt, bqc, qt), (wkt, bkc, kt)):
                    for m in range(2):
                        ps = mp.tile([128, 512], F32, name="proj_ps", tag="mp")
                        for k in range(8):
                            nc.tensor.matmul(
                                ps, lhsT=w_sb[:, k, m * 128:(m + 1) * 128],
                                rhs=xt[:, k, t0:t0 + 512],
                                start=(k == 0), stop=(k == 7))
                        nc.scalar.activation(
                            out=dst[:, m, t0:t0 + 512], in_=ps,
                            func=mybir.ActivationFunctionType.Identity,
                            bias=b_c[:, m:m + 1], scale=1.0)
                # V: out [t 128, dv 256] per 128-subtile
                for tt in range(4):
                    g = 4 * ts + tt
                    ps = mp.tile([128, 256], F32, name="v_ps", tag="mp")
                    for k in range(8):
                        nc.tensor.matmul(
                            ps, lhsT=xt[:, k, g * 128:(g + 1) * 128],
                            rhs=wvt[:, k, :], start=(k == 0), stop=False)
                    nc.tensor.matmul(
                        ps, lhsT=ones_row[0:1, 0:128], rhs=bv_r[0:1, :],
                        start=False, stop=True)
                    # scatter into vaug: head h -> cols [128h+64, 128h+128)
                    nc.scalar.activation(
                        out=vaug[:, g, :].rearrange("p (h c) -> p h c", h=NH)[:, :, HD:],
                        in_=ps.rearrange("p (h c) -> p h c", h=NH),
                        func=mybir.ActivationFunctionType.Copy, scale=1.0)

            # ---- attention ----
            for j in range(NJ):
                q0 = j * QB
                nk = 4 * (j + 1)           # k-tiles of 128 (always even)
                for H in range(2):          # head pair (2H, 2H+1)
                    # E[p, kt, hp, q]
                    e_t = ep.tile([128, KT128, 2, QB], BF16, name="e", tag="e")
                    for g0 in range(0, nk, 2):   # group: 2 kt x 2 heads
                        st = sp.tile([128, 4, 512], F32, name="st", tag="st")
                        for dk in range(2):
                            ktile = g0 + dk
                            s = ktile - 4 * j       # >=0 on diag block
                            c0 = 128 * s if s >= 0 else 0
                            for hp in range(2):
                                h = 2 * H + hp
                                r0 = (HD * h) % 128
                                mi = (HD * h) // 128
                                nc.tensor.matmul(
                                    st[:, 2 * dk + hp, c0:512],
                                    lhsT=kt[r0:r0 + HD, mi,
                                            ktile * 128:(ktile + 1) * 128],
                                    rhs=qt[r0:r0 + HD, mi, q0 + c0:q0 + QB],
                                    start=True, stop=True)
                        nc.scalar.activation(
                            out=e_t[:, g0:g0 + 2, :, :], in_=st,
                            func=EXP, scale=1.0)
                    # diagonal masks: one strided op per head covering the 4
                    # diag tiles (col offset advances 128 with kt -> stride
                    # 2*512+128 along the s axis); tri broadcast via stride 0
                    for hp in range(2):
                        dg = bass.AP(
                            tensor=e_t.tensor,
                            offset=e_t[:, 4 * j, hp, 0:1].offset,
                            ap=[e_t.ap[0], [2 * QB + 128, 4], [1, 128]])
                        trb = bass.AP(
                            tensor=tri.tensor, offset=tri.offset,
                            ap=[tri.ap[0], [0, 4], [1, 128]])
                        nc.vector.tensor_mul(dg, dg, trb)
                    # O^T accumulate + normalize per head
                    for hp in range(2):
                        h = 2 * H + hp
                        ot = tp.tile([128, QB], F32, name="ot", tag="ot")
                        for ktile in range(nk):
                            s = ktile - 4 * j
                            c0 = 128 * s if s >= 0 else 0
                            nc.tensor.matmul(
                                ot[:, c0:QB],
                                lhsT=vaug[:, ktile, 128 * h:128 * (h + 1)],
                                rhs=e_t[:, ktile, hp, c0:QB],
                                start=(ktile == 0), stop=(ktile == nk - 1))
                        rec = bp.tile([64, QB], F32, name="rec", tag="rec")
                        nc.vector.reciprocal_approx_fast(rec, ot[0:64, :])
                        r0 = (HD * h) % 128
                        mi = (HD * h) // 128
                        nc.vector.tensor_mul(
                            ot_all[r0:r0 + HD, mi, q0:q0 + QB],
                            ot[64:128, :], rec)
                # ---- wo projection for this q block ----
                for qq in range(4):
                    row = q0 + qq * 128
                    for n in range(2):
                        ps = mp.tile([128, 512], F32, name="wo_ps", tag="mp")
                        for kk in range(2):
                            nc.tensor.matmul(
                                ps, lhsT=ot_all[:, kk, row:row + 128],
                                rhs=wot[:, kk, n * 512:(n + 1) * 512],
                                start=(kk == 0), stop=(kk == 1))
                        ob = op.tile([128, 512], F32, name="ob", tag="ob")
                        nc.vector.tensor_copy(ob, ps)
                        nc.sync.dma_start(
                            out=p_out[row:row + 128, n * 512:(n + 1) * 512],
                            in_=ob)
    nc.compile()
    return nc


def _prep_inputs(x, wq, bq, wk, bk, wv, bv, wo, bo):
    bf = ml_dtypes.bfloat16
    scale = np.float32(1.0 / np.sqrt(HD))
    tri = np.triu(np.ones((128, 128), np.float32)).astype(bf)
    in_maps = []
    for c in range(NCORES):
        b, g = c // 4, c % 4
        sl = slice(DS * g, DS * (g + 1))
        in_maps.append({
            "xt": np.ascontiguousarray(x[b].T).astype(bf),
            "wqt": np.ascontiguousarray(wq.T[:, sl] * scale).astype(bf),
            "wkt": np.ascontiguousarray(wk.T[:, sl]).astype(bf),
            "wvt": np.ascontiguousarray(wv.T[:, sl]).astype(bf),
            "wot": np.ascontiguousarray(wo[:, sl].T).astype(bf),
            "bqc": np.ascontiguousarray(
                (bq[sl] * scale).reshape(2, 128).T).astype(np.float32),
            "bkc": np.ascontiguousarray(
                bk[sl].reshape(2, 128).T).astype(np.float32),
            "bv": bv[sl].astype(bf).reshape(1, DS),
            "tri": tri,
        })
    return in_maps


TRACE = False
TRACE_DIR = None
LAST_RESULT = None


def kernel(x, wq, bq, wk, bk, wv, bv, wo, bo):
    global LAST_RESULT
    x, wq, bq, wk, bk, wv, bv, wo, bo = [
        np.asarray(a, np.float32)
        for a in (x, wq, bq, wk, bk, wv, bv, wo, bo)]
    if "nc" not in _CACHE:
        _CACHE["nc"] = _build()
    nc = _CACHE["nc"]
    in_maps = _prep_inputs(x, wq, bq, wk, bk, wv, bv, wo, bo)
    res = bass_utils.run_bass_kernel_spmd(
        nc, in_maps, core_ids=list(range(NCORES)), trace=TRACE,
        tmpdir=TRACE_DIR)
    LAST_RESULT = res
    out = np.empty((2, T, DM), np.float32)
    for b in range(2):
        acc = res.results[4 * b]["p_out"].astype(np.float32).copy()
        for g in range(1, 4):
            acc += res.results[4 * b + g]["p_out"]
        out[b] = acc + bo
    return out
